# revision 13
# baseline (speedup 1.0000x reference)
"""Bilinear score kernel for TRN2 (8 NeuronCores, data-parallel over batch).

score[b, t, 0] = states[b, t, :] @ W[0] @ context[b, :] + b[0]

Sharding (per spec hint): states/context sharded on B across the 8 cores
(B == 8 -> one batch per core); W and b replicated.

Per-core dataflow:
  1. v = W @ context_b: 8 fused DVE scalar_tensor_tensor ops over natural-
     layout W tiles (i on partitions) -> v as columns vcols[p, c] = v[c*128+p].
  2. vcols -> PE transpose -> vT[8,128] -> 8 tiny SBUF DMAs -> vrow[1,1024]
     -> PE outer-product with a ones row -> vb[128,1024] (v broadcast
     across partitions).
  3. Stream states (16.8 MB) in [128, 4*1024] tiles; one fused DVE
     scalar_tensor_tensor per 1024-chunk computes 128 dot products:
     accum_out[p] = sum_h states_tile[p, h] * vb[p, h].
  4. Score columns -> PE transpose -> +bias -> single output DMA.

DMA ring usage (HWDGE has two FIFO rings, SP + ACT; SWDGE is separate):
  - states stream: SP ring (nc.sync) -- nothing else may block it
  - W + packed constants + output: ACT ring (nc.scalar)
  - vrow gather (waits on compute): SWDGE (nc.gpsimd)

Memory-bound: ~21 MB/core through HBM at ~358 GB/s.
"""

import numpy as np

import concourse.bass as bass
import concourse.tile as tile
from concourse import bacc, mybir
from concourse.bass import ts
from concourse.bass_utils import run_bass_kernel_spmd

B, T, H = 8, 4096, 1024
P = 128          # SBUF partitions
R = 4            # states rows-of-128 per DMA tile -> [128, R*H] = 2 MB tiles
NT = T // (P * R)    # 8 states tiles per core
WR = 1           # W rows-of-128 per DMA tile (small chunks -> early v start)
WT = H // (P * WR)   # 8 W tiles per core
NCOLS = H // P   # 8 v-columns
NCORES = 8

# packed constants layout: [128, 1024 ctx | 1 bias | 128 ident]
CW = H + 1 + P

F32 = mybir.dt.float32

PROFILE = False          # set True (e.g. from test.py) to capture an NTFF trace
LAST_EXEC_NS = None      # filled when PROFILE is True
LAST_RESULTS = None


def _register_ntff_hook():
    """Register the axon NTFF profile hook that the boot shim skips when
    antenv.axon_hooks is absent from the image. Safe no-op on failure."""
    import sys
    import types

    if "antenv.axon_hooks" in sys.modules:
        return True
    try:
        from trn_agent_boot.trn_boot import _ntff_profile_via_ctypes

        hook = _ntff_profile_via_ctypes("/opt/axon/libaxon_pjrt.so")
        if hook is None:
            return False
        mod = types.ModuleType("antenv.axon_hooks")
        mod.get_axon_ntff_profile_hook = lambda: hook
        sys.modules["antenv.axon_hooks"] = mod
        return True
    except Exception:
        return False


def _build_kernel():
    nc = bacc.Bacc(
        "TRN2",
        target_bir_lowering=False,
        debug=False,
        enable_asserts=False,
        num_devices=NCORES,
    )

    states = nc.dram_tensor("states", [T, H], F32, kind="ExternalInput")
    consts = nc.dram_tensor("consts", [P, CW], F32, kind="ExternalInput")
    w = nc.dram_tensor("w", [H, H], F32, kind="ExternalInput")
    out = nc.dram_tensor("scores", [T, 1], F32, kind="ExternalOutput")

    # DRAM views: t = (n*R + r)*P + p  /  i = (d*WR + r)*P + p
    st_ap = states[:, :].rearrange("(n r p) h -> n p r h", r=R, p=P)
    w_ap = w[:, :].rearrange("(d r p) j -> d p r j", r=WR, p=P)
    out_ap = out[:, :].rearrange("(c p) o -> c (p o)", p=P)

    # states tiles taper at the end so the last DVE ops start sooner
    tile_chunks = [4, 4, 4, 4, 4, 4, 4, 2, 1, 1]
    assert sum(tile_chunks) == T // P

    with tile.TileContext(nc) as tc:
        with (
            tc.tile_pool(name="stp", bufs=6) as stp,
            tc.tile_pool(name="wp", bufs=2) as wp,
            tc.tile_pool(name="sm", bufs=1) as sm,
            tc.tile_pool(name="ps", bufs=2, space="PSUM") as ps,
            tc.tile_pool(name="pso", bufs=2, space="PSUM") as pso,
        ):
            # ---- SP-ring FIFO: consts -> W -> states (strict priority) ----
            const_t = sm.tile([P, CW], F32)
            nc.sync.dma_start(const_t[:, :], consts[:, :])
            ctx_t = const_t[:, 0:H]
            bias_t = const_t[:, H : H + 1]
            id_t = const_t[:, H + 1 : H + 1 + P]

            wts = []
            for d in range(WT):
                wt = wp.tile([P, WR * H], F32)
                nc.sync.dma_start(
                    wt[:, :].rearrange("p (r j) -> p r j", r=WR), w_ap[d]
                )
                wts.append(wt)

            st_full = states[:, :].rearrange("(t p) h -> p t h", p=P)
            st_tiles = []
            row0 = 0
            for rc in tile_chunks:
                st = stp.tile([P, rc * H], F32)
                nc.sync.dma_start(
                    st[:, :].rearrange("p (r h) -> p r h", r=rc),
                    st_full[:, row0 : row0 + rc, :],
                )
                st_tiles.append((st, rc, row0))
                row0 += rc

            ones_t = sm.tile([1, P], F32)
            nc.vector.memset(ones_t[:, :], 1.0)
            dummy = sm.tile([P, 1], F32)

            # ---- v = W @ context_b, broadcast per 128-chunk as W arrives ----
            vcols = sm.tile([P, NCOLS], F32)
            vb = sm.tile([P, H], F32)
            for d in range(WT):
                for r in range(WR):
                    c = d * WR + r
                    nc.vector.scalar_tensor_tensor(
                        out=dummy[:, :].broadcast_to((P, H)),
                        in0=wts[d][:, ts(r, H)],
                        scalar=1.0,
                        in1=ctx_t,
                        op0=mybir.AluOpType.mult,
                        op1=mybir.AluOpType.mult,
                        accum_out=vcols[:, c : c + 1],
                    )
                    # column -> row (PE transpose), row -> 128x128 block bcast
                    # copies on ScalarE to keep DVE free for the STTs
                    rT_ps = ps.tile([1, P], F32, tag="rT")
                    nc.tensor.transpose(rT_ps[:, :], vcols[:, c : c + 1], id_t)
                    row_sb = sm.tile([1, P], F32, tag=f"row{c}")
                    nc.scalar.copy(row_sb[:, :], rT_ps[:, :])
                    blk_ps = ps.tile([P, P], F32, tag="blk")
                    nc.tensor.matmul(
                        blk_ps[:, :], ones_t[0:1, :], row_sb[0:1, :],
                        start=True, stop=True,
                    )
                    nc.scalar.copy(vb[:, ts(c, P)], blk_ps[:, :])

            # ---- scores = states_b . v (columns) ----
            cols = sm.tile([P, T // P], F32)
            flushed = 0

            def flush_out(hi):
                nonlocal flushed
                lo = flushed
                if hi <= lo:
                    return
                n = hi - lo
                o_ps = pso.tile([16, P], F32, tag="ops")
                nc.tensor.transpose(o_ps[0:n, :], cols[:, lo:hi], id_t)
                o_sb = sm.tile([16, P], F32, tag=f"osb{lo}")
                nc.vector.tensor_scalar_add(
                    o_sb[0:n, :], o_ps[0:n, :], bias_t[0:n, :]
                )
                nc.scalar.dma_start(out_ap[lo:hi], o_sb[0:n, :])
                flushed = hi

            for st, rc, row0 in st_tiles:
                for r in range(rc):
                    c = row0 + r
                    nc.vector.scalar_tensor_tensor(
                        out=dummy[:, :].broadcast_to((P, H)),
                        in0=st[:, ts(r, H)],
                        scalar=1.0,
                        in1=vb[:, :],
                        op0=mybir.AluOpType.mult,
                        op1=mybir.AluOpType.mult,
                        accum_out=cols[:, c : c + 1],
                    )
                if row0 + rc in (16, 28, 32):
                    flush_out(row0 + rc)

    nc.compile()
    return nc


def kernel(states: np.ndarray, context: np.ndarray, W: np.ndarray, b: np.ndarray) -> np.ndarray:
    global LAST_EXEC_NS, LAST_RESULTS

    states = np.asarray(states, dtype=np.float32)
    context = np.asarray(context, dtype=np.float32)
    w2d = np.ascontiguousarray(np.asarray(W, dtype=np.float32)[0])
    bias = np.float32(np.asarray(b, dtype=np.float32)[0])

    in_maps = []
    for c in range(NCORES):
        consts = np.empty((P, CW), dtype=np.float32)
        consts[:, 0:H] = context[c][None, :]
        consts[:, H] = bias
        consts[:, H + 1 :] = np.eye(P, dtype=np.float32)
        in_maps.append(
            {
                "states": np.ascontiguousarray(states[c]),
                "consts": consts,
                "w": w2d,
            }
        )

    do_trace = PROFILE and _register_ntff_hook()
    nc = _build_kernel()
    res = run_bass_kernel_spmd(
        nc, in_maps, core_ids=list(range(NCORES)), trace=do_trace
    )
    LAST_EXEC_NS = res.exec_time_ns
    LAST_RESULTS = res

    out = np.stack([res.results[c]["scores"] for c in range(NCORES)], axis=0)
    return out.astype(np.float32)


# revision 16
# speedup vs baseline: 1.0683x; 1.0683x over previous
"""Bilinear score kernel for TRN2 (8 NeuronCores, data-parallel over batch).

score[b, t, 0] = states[b, t, :] @ W[0] @ context[b, :] + b[0]

Sharding (per spec hint): states/context sharded on B across the 8 cores
(B == 8 -> one batch per core); W and b replicated.

Per-core dataflow:
  1. v = W @ context_b: 8 fused DVE scalar_tensor_tensor ops over natural-
     layout W tiles (i on partitions) -> v as columns vcols[p, c] = v[c*128+p].
  2. vcols -> PE transpose -> vT[8,128] -> 8 tiny SBUF DMAs -> vrow[1,1024]
     -> PE outer-product with a ones row -> vb[128,1024] (v broadcast
     across partitions).
  3. Stream states (16.8 MB) in [128, 4*1024] tiles; one fused DVE
     scalar_tensor_tensor per 1024-chunk computes 128 dot products:
     accum_out[p] = sum_h states_tile[p, h] * vb[p, h].
  4. Score columns -> PE transpose -> +bias -> single output DMA.

DMA ring usage (HWDGE has two FIFO rings, SP + ACT; SWDGE is separate):
  - states stream: SP ring (nc.sync) -- nothing else may block it
  - W + packed constants + output: ACT ring (nc.scalar)
  - vrow gather (waits on compute): SWDGE (nc.gpsimd)

Memory-bound: ~21 MB/core through HBM at ~358 GB/s.
"""

import numpy as np

import concourse.bass as bass
import concourse.tile as tile
from concourse import bacc, mybir
from concourse.bass import ts
from concourse.bass_utils import run_bass_kernel_spmd

B, T, H = 8, 4096, 1024
P = 128          # SBUF partitions
R = 4            # states rows-of-128 per DMA tile -> [128, R*H] = 2 MB tiles
NT = T // (P * R)    # 8 states tiles per core
WR = 1           # W rows-of-128 per DMA tile (small chunks -> early v start)
WT = H // (P * WR)   # 8 W tiles per core
NCOLS = H // P   # 8 v-columns
NCORES = 8

# packed constants layout: [128, 1024 ctx | 1 bias | 128 ident]
CW = H + 1 + P

F32 = mybir.dt.float32

PROFILE = False          # set True (e.g. from test.py) to capture an NTFF trace
LAST_EXEC_NS = None      # filled when PROFILE is True
LAST_RESULTS = None


def _register_ntff_hook():
    """Register the axon NTFF profile hook that the boot shim skips when
    antenv.axon_hooks is absent from the image. Safe no-op on failure."""
    import sys
    import types

    if "antenv.axon_hooks" in sys.modules:
        return True
    try:
        from trn_agent_boot.trn_boot import _ntff_profile_via_ctypes

        hook = _ntff_profile_via_ctypes("/opt/axon/libaxon_pjrt.so")
        if hook is None:
            return False
        mod = types.ModuleType("antenv.axon_hooks")
        mod.get_axon_ntff_profile_hook = lambda: hook
        sys.modules["antenv.axon_hooks"] = mod
        return True
    except Exception:
        return False


def _build_kernel():
    nc = bacc.Bacc(
        "TRN2",
        target_bir_lowering=False,
        debug=False,
        enable_asserts=False,
        num_devices=NCORES,
    )

    states = nc.dram_tensor("states", [T, H], F32, kind="ExternalInput")
    consts = nc.dram_tensor("consts", [P, CW], F32, kind="ExternalInput")
    w = nc.dram_tensor("w", [H, H], F32, kind="ExternalInput")
    out = nc.dram_tensor("scores", [T, 1], F32, kind="ExternalOutput")

    # DRAM views: t = (n*R + r)*P + p  /  i = (d*WR + r)*P + p
    st_ap = states[:, :].rearrange("(n r p) h -> n p r h", r=R, p=P)
    w_ap = w[:, :].rearrange("(d r p) j -> d p r j", r=WR, p=P)
    out_ap = out[:, :].rearrange("(c p) o -> c (p o)", p=P)

    # states tiles taper at the end so the last DVE ops start sooner
    tile_chunks = [4, 4, 4, 4, 4, 4, 4, 2, 1, 1]
    assert sum(tile_chunks) == T // P

    with tile.TileContext(nc) as tc:
        with (
            tc.tile_pool(name="stp", bufs=6) as stp,
            tc.tile_pool(name="wp", bufs=WT) as wp,
            tc.tile_pool(name="sm", bufs=1) as sm,
            tc.tile_pool(name="ps", bufs=2, space="PSUM") as ps,
            tc.tile_pool(name="pso", bufs=2, space="PSUM") as pso,
        ):
            # ---- SP-ring FIFO: consts -> W -> states (strict priority) ----
            const_t = sm.tile([P, CW], F32)
            nc.sync.dma_start(const_t[:, :], consts[:, :])
            ctx_t = const_t[:, 0:H]
            bias_t = const_t[:, H : H + 1]
            id_t = const_t[:, H + 1 : H + 1 + P]

            wts = []
            for d in range(WT):
                wt = wp.tile([P, WR * H], F32)
                nc.sync.dma_start(
                    wt[:, :].rearrange("p (r j) -> p r j", r=WR), w_ap[d]
                )
                wts.append(wt)

            st_full = states[:, :].rearrange("(t p) h -> p t h", p=P)
            st_tiles = []
            row0 = 0
            for rc in tile_chunks:
                st = stp.tile([P, rc * H], F32)
                nc.sync.dma_start(
                    st[:, :].rearrange("p (r h) -> p r h", r=rc),
                    st_full[:, row0 : row0 + rc, :],
                )
                st_tiles.append((st, rc, row0))
                row0 += rc

            ones_t = sm.tile([1, P], F32)
            nc.vector.memset(ones_t[:, :], 1.0)
            dummy = sm.tile([P, 1], F32)

            # ---- v = W @ context_b, broadcast per 128-chunk as W arrives ----
            vcols = sm.tile([P, NCOLS], F32)
            vb = sm.tile([P, H], F32)
            for d in range(WT):
                for r in range(WR):
                    c = d * WR + r
                    nc.vector.scalar_tensor_tensor(
                        out=dummy[:, :].broadcast_to((P, H)),
                        in0=wts[d][:, ts(r, H)],
                        scalar=1.0,
                        in1=ctx_t,
                        op0=mybir.AluOpType.mult,
                        op1=mybir.AluOpType.mult,
                        accum_out=vcols[:, c : c + 1],
                    )
                    # column -> row (PE transpose), row -> 128x128 block bcast
                    # copies on ScalarE to keep DVE free for the STTs
                    rT_ps = ps.tile([1, P], F32, tag="rT")
                    nc.tensor.transpose(rT_ps[:, :], vcols[:, c : c + 1], id_t)
                    row_sb = sm.tile([1, P], F32, tag=f"row{c}")
                    nc.scalar.copy(row_sb[:, :], rT_ps[:, :])
                    blk_ps = ps.tile([P, P], F32, tag="blk")
                    nc.tensor.matmul(
                        blk_ps[:, :], ones_t[0:1, :], row_sb[0:1, :],
                        start=True, stop=True,
                    )
                    nc.scalar.copy(vb[:, ts(c, P)], blk_ps[:, :])

            # ---- scores = states_b . v (columns) ----
            cols = sm.tile([P, T // P], F32)
            flushed = 0

            def flush_out(hi):
                nonlocal flushed
                lo = flushed
                if hi <= lo:
                    return
                n = hi - lo
                o_ps = pso.tile([16, P], F32, tag="ops")
                nc.tensor.transpose(o_ps[0:n, :], cols[:, lo:hi], id_t)
                o_sb = sm.tile([16, P], F32, tag=f"osb{lo}")
                nc.scalar.activation(
                    o_sb[0:n, :], o_ps[0:n, :],
                    mybir.ActivationFunctionType.Identity, bias=bias_t[0:n, :],
                )
                nc.scalar.dma_start(out_ap[lo:hi], o_sb[0:n, :])
                flushed = hi

            for st, rc, row0 in st_tiles:
                for r in range(rc):
                    c = row0 + r
                    nc.vector.scalar_tensor_tensor(
                        out=dummy[:, :].broadcast_to((P, H)),
                        in0=st[:, ts(r, H)],
                        scalar=1.0,
                        in1=vb[:, :],
                        op0=mybir.AluOpType.mult,
                        op1=mybir.AluOpType.mult,
                        accum_out=cols[:, c : c + 1],
                    )
                if row0 + rc in (16, 28, 32):
                    flush_out(row0 + rc)

    nc.compile()
    return nc


def kernel(states: np.ndarray, context: np.ndarray, W: np.ndarray, b: np.ndarray) -> np.ndarray:
    global LAST_EXEC_NS, LAST_RESULTS

    states = np.asarray(states, dtype=np.float32)
    context = np.asarray(context, dtype=np.float32)
    w2d = np.ascontiguousarray(np.asarray(W, dtype=np.float32)[0])
    bias = np.float32(np.asarray(b, dtype=np.float32)[0])

    in_maps = []
    for c in range(NCORES):
        consts = np.empty((P, CW), dtype=np.float32)
        consts[:, 0:H] = context[c][None, :]
        consts[:, H] = bias
        consts[:, H + 1 :] = np.eye(P, dtype=np.float32)
        in_maps.append(
            {
                "states": np.ascontiguousarray(states[c]),
                "consts": consts,
                "w": w2d,
            }
        )

    do_trace = PROFILE and _register_ntff_hook()
    nc = _build_kernel()
    res = run_bass_kernel_spmd(
        nc, in_maps, core_ids=list(range(NCORES)), trace=do_trace
    )
    LAST_EXEC_NS = res.exec_time_ns
    LAST_RESULTS = res

    out = np.stack([res.results[c]["scores"] for c in range(NCORES)], axis=0)
    return out.astype(np.float32)
